# revision 46
# baseline (speedup 1.0000x reference)
"""GQA forward (B=2,N=2048,D=2048,H=32,KV=8,DH=64, causal) on 8 trn2 cores.

Sharding: 2-way data parallel over batch x 4-way tensor parallel over heads
(each core: 8 q-heads = 2 kv-heads, keeping group structure). Row-parallel
out-proj; the all-reduce over the 4 TP shards (+ bias) happens on host at
gather time.

Device kernel (per core), all PE matmuls in bf16 (PSUM accum fp32).
Half-sized matmuls stream at the slow PE clock on trn2, so every matmul is
padded to the full 128-partition contraction / 128 stationary columns:

  phase 1: streaming projections from xT (host-pretransposed bf16), input
           DMAs interleaved across queues so the first matmul starts ~2us
           in. K^T is stored per kv head with the other head's 64 rows
           zeroed (the zeros annihilate the other head's q rows, keeping
           K=128). V^T -> V via bf16 PE transposes deferred into the next
           q-block's matmul stream.
  phase 2: causal attention per head in S^T orientation, q-blocks outer,
           heads inner. Scores for two 128-key blocks share a double-bank
           psum tile and a single exp (ACT, folded 1/sqrt(dh) scale, bf16
           out). Triangle mask multiply on diagonal blocks only. ctx^T
           accumulates in psum; vaug's ones column gives the softmax
           denominator on partition 64 (cols 65:128 are pad). The
           denominator is broadcast with a full-K ones matmul against a
           persistent zeroed row tile, then fast-reciprocal + DVE
           multiply. ctx pairs trail their scores pair by two so the
           in-order PE queue never waits on ACT.
  phase 2.5: out-projection row tiles of q-block nb are interleaved two
           (nt, ob) units per head-block into q-block nb+1's attention,
           overlapping out-proj PE work with ACT exp time.
  phase 3: the final q-block's out-proj tail, stored bf16 per chunk
           (host upcasts and all-reduces the TP shards).
"""
import os
import sys
from collections import deque

import numpy as np

if "/opt/trn_rl_repo" not in sys.path:
    sys.path.insert(0, "/opt/trn_rl_repo")

import ml_dtypes

import concourse.bacc as bacc
import concourse.tile as tile
from concourse import mybir
from concourse.bass_utils import run_bass_kernel_spmd
from concourse.masks import make_identity

F32 = mybir.dt.float32
BF16 = mybir.dt.bfloat16
EXP = mybir.ActivationFunctionType.Exp
BF = ml_dtypes.bfloat16

B, N, D = 2, 2048, 2048
H, KV, DH = 32, 8, 64
G = H // KV                      # 4 q-heads per kv head
HPC, KVPC = 8, 2                 # heads / kv-heads per core
DQ = HPC * DH                    # 512 per-core q projection width
NT = N // 128                    # 16 row tiles
NBW = 512                        # q-block width for attention
NB = N // NBW                    # 4 q-blocks
DC = D // 128                    # 16 contraction chunks
VW2 = 128                        # padded vaug stride per kv head

_CACHED = {}


def _build():
    nc = bacc.Bacc("TRN2", target_bir_lowering=False, debug=False, num_devices=8)

    xT = nc.dram_tensor("xT", [D, N], BF16, kind="ExternalInput")
    Wq = nc.dram_tensor("Wq", [D, DQ], BF16, kind="ExternalInput")
    Wk = nc.dram_tensor("Wk", [D, KVPC * DH], BF16, kind="ExternalInput")
    Wv = nc.dram_tensor("Wv", [D, KVPC * DH], BF16, kind="ExternalInput")
    Wo = nc.dram_tensor("Wo", [DQ, D], BF16, kind="ExternalInput")
    OUT = nc.dram_tensor("out", [N, D], BF16, kind="ExternalOutput")

    with tile.TileContext(nc) as tc:
        with (
            tc.tile_pool(name="persist", bufs=1) as pp,
            tc.tile_pool(name="wbig", bufs=16) as wbig,
            tc.tile_pool(name="wkv", bufs=16) as wkvp,
            tc.tile_pool(name="xs", bufs=6) as xsp,
            tc.tile_pool(name="vt", bufs=2) as vtp,
            tc.tile_pool(name="pt", bufs=10) as ptp,
            tc.tile_pool(name="outs", bufs=2) as outp,
            tc.tile_pool(name="small", bufs=3) as smp,
            tc.tile_pool(name="ps", bufs=4, space="PSUM") as psp,
            tc.tile_pool(name="ps2", bufs=2, space="PSUM") as psp2,
        ):
            # ---- persistent sbuf state ----
            ident = pp.tile([128, 128], BF16, tag="ident")
            tri = pp.tile([128, 128], BF16, tag="tri")

            qt = [pp.tile([128, N], BF16, tag=f"qt{s}", name=f"qt{s}")
                  for s in range(4)]
            # K^T per kv head padded to the full 128 contraction rows; the
            # other kv head's 64 rows stay zero.
            ktp = [pp.tile([128, N], BF16, tag=f"kt{v}", name=f"kt{v}")
                   for v in range(KVPC)]
            for v in range(KVPC):
                nc.vector.memset(ktp[v][:], 0.0)
            # vaug (stride VW2=128 per kv head): [V (64) | ones (64)]; col 64
            # puts the softmax row-sum on psum partition 64 of the ctx
            # matmul, cols 65:128 pad the stationary to 128 (their psum
            # partitions are never read). memset once; only V columns get
            # overwritten.
            vaug = [pp.tile([128, KVPC * VW2], BF16, tag=f"va{m}", name=f"va{m}")
                    for m in range(NT)]
            for m in range(NT):
                nc.vector.memset(vaug[m][:], 1.0)
            ctxT = [pp.tile([128, N], BF16, tag=f"ct{j}", name=f"ct{j}")
                    for j in range(4)]
            # full-K denominator broadcast: onesP row 64 is ones, the lr
            # tiles are zero everywhere except row 64 (written per norm),
            # so the K=128 ones matmul reproduces the K=1 broadcast at the
            # fast clock with no garbage terms.
            onesP = pp.tile([128, DH], BF16, tag="onesP")
            nc.vector.memset(onesP[:], 0.0)
            nc.vector.memset(onesP[DH:DH + 1, :], 1.0)
            lr = [pp.tile([128, NBW], BF16, tag=f"lr{i}", name=f"lr{i}")
                  for i in range(2)]
            for i in range(2):
                nc.vector.memset(lr[i][:], 0.0)

            # ---- weight loads spread over the idle queues (and their
            # DMA engines): wk on scalar, wv+wq interleaved on gpsimd,
            # wo after them; the x stream has sync to itself ----
            wk_sb, wv_sb, wq_sb = [], [], []
            for dc in range(DC):
                t = wkvp.tile([128, KVPC * DH], BF16, tag="wk")
                nc.scalar.dma_start(out=t[:], in_=Wk[dc * 128:(dc + 1) * 128, :])
                wk_sb.append(t)
            for dc in range(DC):
                t = wkvp.tile([128, KVPC * DH], BF16, tag="wv")
                nc.gpsimd.dma_start(out=t[:], in_=Wv[dc * 128:(dc + 1) * 128, :])
                wv_sb.append(t)
                t = wbig.tile([128, DQ], BF16, tag="w")
                nc.gpsimd.dma_start(out=t[:], in_=Wq[dc * 128:(dc + 1) * 128, :])
                wq_sb.append(t)
            wo_sb = {}
            for j in range(4):
                for ob in range(4):
                    t = wbig.tile([128, NBW], BF16, tag="wo")
                    nc.gpsimd.dma_start(
                        out=t[:],
                        in_=Wo[j * 128:(j + 1) * 128, ob * NBW:(ob + 1) * NBW])
                    wo_sb[(j, ob)] = t

            # identity / triangle mask setup rides the gpsimd queue after
            # the weight DMA issues (neither is needed until ~100us in)
            make_identity(nc, ident[:])
            nc.gpsimd.memset(tri[:], 1.0)
            # lower-triangle-in-column-sense mask: tri[r, j] = 1 if j >= r
            nc.gpsimd.affine_select(
                out=tri[:], in_=tri[:],
                compare_op=mybir.AluOpType.is_ge,
                fill=0.0, base=0,
                pattern=[[1, 128]],
                channel_multiplier=-1,
            )

            def emit_proj(nb):
                ncol = slice(nb * NBW, (nb + 1) * NBW)
                q_ps = [psp.tile([128, NBW], F32, tag="ps", name=f"qps{_}")
                        for _ in range(4)]
                kv_ps = psp2.tile([128, 2 * NBW], F32, tag="spair", name="kvps")
                for dc in range(DC):
                    xs = xsp.tile([128, NBW], BF16, tag="xs")
                    nc.sync.dma_start(out=xs[:],
                                      in_=xT[dc * 128:(dc + 1) * 128, ncol])
                    st, sp = dc == 0, dc == DC - 1
                    for s in range(4):
                        nc.tensor.matmul(q_ps[s][:],
                                         wq_sb[dc][:, s * 128:(s + 1) * 128],
                                         xs[:], start=st, stop=sp)
                    nc.tensor.matmul(kv_ps[:, 0:NBW], wk_sb[dc][:], xs[:],
                                     start=st, stop=sp)
                    nc.tensor.matmul(kv_ps[:, NBW:2 * NBW], wv_sb[dc][:],
                                     xs[:], start=st, stop=sp)
                    if dc == 0:
                        flush_tr()
                # vts copy first: the deferred transposes depend on it
                vts = vtp.tile([128, NBW], BF16, tag="vts")
                nc.vector.tensor_copy(vts[:], kv_ps[:, NBW:2 * NBW])
                for s in range(4):
                    nc.vector.tensor_copy(qt[s][:, ncol], q_ps[s][:])
                nc.vector.tensor_copy(ktp[0][0:64, ncol],
                                      kv_ps[0:64, 0:NBW])
                nc.vector.tensor_copy(ktp[1][64:128, ncol],
                                      kv_ps[64:128, 0:NBW])

                def _tr(vts=vts, nb=nb):
                    tq = psp2.tile([128, 2 * NBW], BF16, tag="spair",
                                   name="tq")
                    for i in range(4):
                        nc.tensor.transpose(tq[:, i * 128:(i + 1) * 128],
                                            vts[:, i * 128:(i + 1) * 128],
                                            ident[:])
                    for i in range(4):
                        mt = nb * 4 + i
                        nc.vector.tensor_copy(vaug[mt][:, 0:DH],
                                              tq[:, i * 128:i * 128 + DH])
                        nc.vector.tensor_copy(
                            vaug[mt][:, VW2:VW2 + DH],
                            tq[:, i * 128 + DH:i * 128 + 2 * DH])
                nonlocal_pend_tr[0] = _tr

            nonlocal_pend_tr = [None]

            def flush_tr():
                if nonlocal_pend_tr[0] is not None:
                    nonlocal_pend_tr[0]()
                    nonlocal_pend_tr[0] = None

            # ---- attention (+ interleaved out-proj units) ----
            scale = 1.0 / np.sqrt(DH)
            norm_idx = [0]

            def emit_norm(c_ps, j, par, q0):
                # ctx^T rows /= row 64 (the ones-col sums): write the sums
                # into the zeroed row tile, broadcast to partitions 0:64
                # with the full-K ones matmul, fast reciprocal, multiply.
                # Engines cannot shift partitions, so the odd-parity half
                # goes through a small sbuf->sbuf DMA into ctxT 64:128.
                lrt = lr[norm_idx[0] % 2]
                norm_idx[0] += 1
                nc.vector.tensor_copy(lrt[DH:DH + 1, :], c_ps[DH:DH + 1, :])
                rb_ps = psp.tile([DH, NBW], F32, tag="ps", name="rbps")
                nc.tensor.matmul(rb_ps[:], onesP[:], lrt[:],
                                 start=True, stop=True)
                rb = smp.tile([DH, NBW], F32, tag="rb", name="rb")
                nc.vector.reciprocal_approx_fast(rb[:], rb_ps[:])
                if par == 0:
                    nc.vector.tensor_mul(ctxT[j][0:DH, q0:q0 + NBW],
                                         c_ps[0:DH, :], rb[:])
                else:
                    tmp = smp.tile([DH, NBW], BF16, tag="ctmp", name="ctmp")
                    nc.vector.tensor_mul(tmp[:], c_ps[0:DH, :], rb[:])
                    nc.sync.dma_start(out=ctxT[j][DH:2 * DH, q0:q0 + NBW],
                                      in_=tmp[:])

            o_sb_of = {}
            pend_ocopy = [None]

            def emit_ounit(nt, ob):
                # one (row-tile, out-block) unit of the out projection;
                # the psum->sbuf copy + store are deferred one unit so
                # attention DVE work is never queued behind them
                if nt not in o_sb_of:
                    o_sb_of[nt] = outp.tile([128, D], BF16, tag="osb",
                                            name="osb")
                o_sb = o_sb_of[nt]
                o_ps = psp.tile([128, NBW], F32, tag="ps", name="ops")
                for j in range(4):
                    nc.tensor.matmul(o_ps[:],
                                     ctxT[j][:, nt * 128:(nt + 1) * 128],
                                     wo_sb[(j, ob)][:],
                                     start=(j == 0), stop=(j == 3))
                if pend_ocopy[0] is not None:
                    pend_ocopy[0]()

                def _ocopy(o_sb=o_sb, o_ps=o_ps, nt=nt, ob=ob):
                    nc.vector.tensor_copy(o_sb[:, ob * NBW:(ob + 1) * NBW],
                                          o_ps[:])
                    nc.sync.dma_start(
                        out=OUT[nt * 128:(nt + 1) * 128,
                                ob * NBW:(ob + 1) * NBW],
                        in_=o_sb[:, ob * NBW:(ob + 1) * NBW])
                pend_ocopy[0] = _ocopy

            fin_state = [None]
            pend_units = deque()

            def flush_fin():
                if fin_state[0] is not None:
                    fin_state[0]()
                    fin_state[0] = None

            def emit_attn_block(hh, nb):
                    kv, g = hh // G, hh % G
                    j, par = hh // 2, hh % 2
                    q0 = nb * NBW
                    c_ps = psp.tile([128, NBW], F32, tag="ps", name="cps")
                    vcol = slice(kv * VW2, (kv + 1) * VW2)
                    n_mb = 4 * nb + 4
                    pend = deque()
                    for pr in range(n_mb // 2):
                        s_pair = psp2.tile([128, 2 * NBW], F32, tag="spair",
                                           name="sp")
                        widths = []
                        for half in (0, 1):
                            mb = 2 * pr + half
                            m0 = mb * 128
                            off = max(0, m0 - q0)
                            w = NBW - off
                            widths.append((mb, off, w))
                            nc.tensor.matmul(
                                s_pair[:, half * NBW:half * NBW + w],
                                ktp[kv][:, m0:m0 + 128],
                                qt[g][:, q0 + off:q0 + NBW],
                                start=True, stop=True)
                        p_pair = ptp.tile([128, 2 * NBW], BF16, tag="pt",
                                          name="pt")
                        ew = NBW + widths[1][2]
                        nc.scalar.activation(p_pair[:, 0:ew], s_pair[:, 0:ew],
                                             EXP, scale=float(scale))
                        for half in (0, 1):
                            mb, off, w = widths[half]
                            if mb >= 4 * nb:  # diagonal: triangle mask
                                nc.vector.tensor_mul(
                                    p_pair[:, half * NBW:half * NBW + 128],
                                    p_pair[:, half * NBW:half * NBW + 128],
                                    tri[:])

                        def _ctxpair(c_ps=c_ps, p_pair=p_pair, widths=widths,
                                     vcol=vcol, n_mb=n_mb):
                            for half in (0, 1):
                                mb, off, w = widths[half]
                                nc.tensor.matmul(
                                    c_ps[:, off:NBW],
                                    vaug[mb][:, vcol],
                                    p_pair[:, half * NBW:half * NBW + w],
                                    start=(mb == 0), stop=(mb == n_mb - 1))
                        pend.append(_ctxpair)
                        if pr == 1:
                            flush_fin()

                    def _fin(pend=pend, c_ps=c_ps, j=j, par=par, q0=q0):
                        while pend:
                            pend.popleft()()
                        emit_norm(c_ps, j, par, q0)
                    fin_state[0] = _fin

                    # interleave two out-proj units of the previous q-block
                    for _ in range(2):
                        if pend_units:
                            emit_ounit(*pend_units.popleft())

            # ---- schedule: projections first, then attention q-blocks
            # with the previous block's out-proj units interleaved two per
            # head-block (they fill the PE while ACT works through exps).
            for nb in range(NB):
                emit_proj(nb)
            for nb in range(NB):
                if nb == NB - 1:
                    # the last q-block's V transposes are only needed here;
                    # emitting them now costs nothing (vts long since ready)
                    flush_tr()
                for hh in range(HPC):
                    emit_attn_block(hh, nb)
                # all heads of nb emitted; its out-proj units become
                # available once the last fin lands (next block, pr==1)
                pend_units.extend((4 * nb + i, ob)
                                  for i in range(4) for ob in range(4))
            flush_fin()
            while pend_units:
                emit_ounit(*pend_units.popleft())
            if pend_ocopy[0] is not None:
                pend_ocopy[0]()

    nc.compile()
    return nc


def kernel(x, Wq, Wk, Wv, Wo, bo):
    x = np.asarray(x, dtype=np.float32)
    Wq = np.asarray(Wq, dtype=np.float32)
    Wk = np.asarray(Wk, dtype=np.float32)
    Wv = np.asarray(Wv, dtype=np.float32)
    Wo = np.asarray(Wo, dtype=np.float32)
    bo = np.asarray(bo, dtype=np.float32)

    if "nc" not in _CACHED:
        _CACHED["nc"] = _build()
    nc = _CACHED["nc"]

    xTb = [x[b].T.astype(BF) for b in range(B)]
    wk_t = [Wk[:, t * 128:(t + 1) * 128].astype(BF) for t in range(4)]
    wv_t = [Wv[:, t * 128:(t + 1) * 128].astype(BF) for t in range(4)]
    wo_t = [Wo[t * DQ:(t + 1) * DQ, :].astype(BF) for t in range(4)]
    wq_t = []
    for t in range(4):
        # q slab s holds [kv-head 2t head g=s | kv-head 2t+1 head g=s]
        qcols = []
        for s in range(4):
            for kvl in range(KVPC):
                h = (2 * t + kvl) * G + s
                qcols.append(Wq[:, h * DH:(h + 1) * DH])
        wq_t.append(np.concatenate(qcols, axis=1).astype(BF))

    in_maps = []
    for c in range(8):
        b, t = c // 4, c % 4
        in_maps.append({"xT": xTb[b], "Wq": wq_t[t], "Wk": wk_t[t],
                        "Wv": wv_t[t], "Wo": wo_t[t]})

    trace = bool(int(os.environ.get("GQA_TRACE", "0")))
    kwargs = {}
    if trace:
        import tempfile
        td = os.environ.get("GQA_TRACE_DIR") or tempfile.mkdtemp(prefix="gqa_")
        kwargs = dict(trace=True, tmpdir=td)
    res = run_bass_kernel_spmd(nc, in_maps, list(range(8)), **kwargs)
    _CACHED["last_result"] = res

    out = np.empty((B, N, D), dtype=np.float32)
    for b in range(B):
        acc = res.results[4 * b]["out"].astype(np.float32)
        for t in range(1, 4):
            acc = acc + res.results[4 * b + t]["out"].astype(np.float32)
        out[b] = acc + bo[None, :]
    return out


# revision 48
# speedup vs baseline: 1.0115x; 1.0115x over previous
"""GQA forward (B=2,N=2048,D=2048,H=32,KV=8,DH=64, causal) on 8 trn2 cores.

Sharding: 2-way data parallel over batch x 4-way tensor parallel over heads
(each core: 8 q-heads = 2 kv-heads, keeping group structure). Row-parallel
out-proj; the all-reduce over the 4 TP shards (+ bias) happens on host at
gather time.

Device kernel (per core), all PE matmuls in bf16 (PSUM accum fp32).
Half-sized matmuls stream at the slow PE clock on trn2, so every matmul is
padded to the full 128-partition contraction / 128 stationary columns:

  phase 1: streaming projections from xT (host-pretransposed bf16), input
           DMAs interleaved across queues so the first matmul starts ~2us
           in. K^T is stored per kv head with the other head's 64 rows
           zeroed (the zeros annihilate the other head's q rows, keeping
           K=128). V^T -> V via bf16 PE transposes deferred into the next
           q-block's matmul stream.
  phase 2: causal attention per head in S^T orientation, q-blocks outer,
           heads inner. Scores for two 128-key blocks share a double-bank
           psum tile and a single exp (ACT, folded 1/sqrt(dh) scale, bf16
           out). Triangle mask multiply on diagonal blocks only. ctx^T
           accumulates in psum; vaug's ones column gives the softmax
           denominator on partition 64 (cols 65:128 are pad). The
           denominator is broadcast with a full-K ones matmul against a
           persistent zeroed row tile, then fast-reciprocal + DVE
           multiply. ctx pairs trail their scores pair by two so the
           in-order PE queue never waits on ACT.
  phase 2.5: out-projection row tiles of q-block nb are interleaved two
           (nt, ob) units per head-block into q-block nb+1's attention,
           overlapping out-proj PE work with ACT exp time.
  phase 3: the final q-block's out-proj tail, stored bf16 per chunk
           (host upcasts and all-reduces the TP shards).
"""
import os
import sys
from collections import deque

import numpy as np

if "/opt/trn_rl_repo" not in sys.path:
    sys.path.insert(0, "/opt/trn_rl_repo")

import ml_dtypes

import concourse.bacc as bacc
import concourse.tile as tile
from concourse import mybir
from concourse.bass_utils import run_bass_kernel_spmd
from concourse.masks import make_identity

F32 = mybir.dt.float32
BF16 = mybir.dt.bfloat16
EXP = mybir.ActivationFunctionType.Exp
BF = ml_dtypes.bfloat16

B, N, D = 2, 2048, 2048
H, KV, DH = 32, 8, 64
G = H // KV                      # 4 q-heads per kv head
HPC, KVPC = 8, 2                 # heads / kv-heads per core
DQ = HPC * DH                    # 512 per-core q projection width
NT = N // 128                    # 16 row tiles
NBW = 512                        # q-block width for attention
NB = N // NBW                    # 4 q-blocks
DC = D // 128                    # 16 contraction chunks
VW2 = 128                        # padded vaug stride per kv head

_CACHED = {}


def _build():
    nc = bacc.Bacc("TRN2", target_bir_lowering=False, debug=False, num_devices=8)

    xT = nc.dram_tensor("xT", [D, N], BF16, kind="ExternalInput")
    Wq = nc.dram_tensor("Wq", [D, DQ], BF16, kind="ExternalInput")
    Wk = nc.dram_tensor("Wk", [D, KVPC * DH], BF16, kind="ExternalInput")
    Wv = nc.dram_tensor("Wv", [D, KVPC * DH], BF16, kind="ExternalInput")
    Wo = nc.dram_tensor("Wo", [DQ, D], BF16, kind="ExternalInput")
    OUT = nc.dram_tensor("out", [N, D], BF16, kind="ExternalOutput")

    with tile.TileContext(nc) as tc:
        with (
            tc.tile_pool(name="persist", bufs=1) as pp,
            tc.tile_pool(name="wbig", bufs=16) as wbig,
            tc.tile_pool(name="wkv", bufs=16) as wkvp,
            tc.tile_pool(name="xs", bufs=6) as xsp,
            tc.tile_pool(name="vt", bufs=2) as vtp,
            tc.tile_pool(name="pt", bufs=10) as ptp,
            tc.tile_pool(name="outs", bufs=2) as outp,
            tc.tile_pool(name="small", bufs=3) as smp,
            tc.tile_pool(name="ps", bufs=4, space="PSUM") as psp,
            tc.tile_pool(name="ps2", bufs=2, space="PSUM") as psp2,
        ):
            # ---- persistent sbuf state ----
            ident = pp.tile([128, 128], BF16, tag="ident")
            make_identity(nc, ident[:])
            # lower-triangle-in-column-sense mask: mask[r, j] = 1 if j >= r
            tri = pp.tile([128, 128], BF16, tag="tri")
            nc.gpsimd.memset(tri[:], 1.0)
            nc.gpsimd.affine_select(
                out=tri[:], in_=tri[:],
                compare_op=mybir.AluOpType.is_ge,
                fill=0.0, base=0,
                pattern=[[1, 128]],
                channel_multiplier=-1,
            )

            qt = [pp.tile([128, N], BF16, tag=f"qt{s}", name=f"qt{s}")
                  for s in range(4)]
            # K^T per kv head padded to the full 128 contraction rows; the
            # other kv head's 64 rows stay zero.
            ktp = [pp.tile([128, N], BF16, tag=f"kt{v}", name=f"kt{v}")
                   for v in range(KVPC)]
            for v in range(KVPC):
                nc.vector.memset(ktp[v][:], 0.0)
            # vaug (stride VW2=128 per kv head): [V (64) | ones (64)]; col 64
            # puts the softmax row-sum on psum partition 64 of the ctx
            # matmul, cols 65:128 pad the stationary to 128 (their psum
            # partitions are never read). memset once; only V columns get
            # overwritten.
            vaug = [pp.tile([128, KVPC * VW2], BF16, tag=f"va{m}", name=f"va{m}")
                    for m in range(NT)]
            for m in range(NT):
                nc.vector.memset(vaug[m][:], 1.0)
            ctxT = [pp.tile([128, N], BF16, tag=f"ct{j}", name=f"ct{j}")
                    for j in range(4)]
            # full-K denominator broadcast: onesP row 64 is ones, the lr
            # tiles are zero everywhere except row 64 (written per norm),
            # so the K=128 ones matmul reproduces the K=1 broadcast at the
            # fast clock with no garbage terms.
            onesP = pp.tile([128, DH], BF16, tag="onesP")
            nc.vector.memset(onesP[:], 0.0)
            nc.vector.memset(onesP[DH:DH + 1, :], 1.0)
            lr = [pp.tile([128, NBW], BF16, tag=f"lr{i}", name=f"lr{i}")
                  for i in range(2)]
            for i in range(2):
                nc.vector.memset(lr[i][:], 0.0)

            # ---- weight loads spread over the idle queues (and their
            # DMA engines): wk on scalar, wv+wq interleaved on gpsimd,
            # wo after them; the x stream has sync to itself ----
            wk_sb, wv_sb, wq_sb = [], [], []
            for dc in range(DC):
                t = wkvp.tile([128, KVPC * DH], BF16, tag="wk")
                nc.scalar.dma_start(out=t[:], in_=Wk[dc * 128:(dc + 1) * 128, :])
                wk_sb.append(t)
            for dc in range(DC):
                t = wkvp.tile([128, KVPC * DH], BF16, tag="wv")
                nc.gpsimd.dma_start(out=t[:], in_=Wv[dc * 128:(dc + 1) * 128, :])
                wv_sb.append(t)
                t = wbig.tile([128, DQ], BF16, tag="w")
                nc.gpsimd.dma_start(out=t[:], in_=Wq[dc * 128:(dc + 1) * 128, :])
                wq_sb.append(t)
            wo_sb = {}
            for j in range(4):
                for ob in range(4):
                    t = wbig.tile([128, NBW], BF16, tag="wo")
                    nc.gpsimd.dma_start(
                        out=t[:],
                        in_=Wo[j * 128:(j + 1) * 128, ob * NBW:(ob + 1) * NBW])
                    wo_sb[(j, ob)] = t

            def emit_proj(nb):
                ncol = slice(nb * NBW, (nb + 1) * NBW)
                q_ps = [psp.tile([128, NBW], F32, tag="ps", name=f"qps{_}")
                        for _ in range(4)]
                kv_ps = psp2.tile([128, 2 * NBW], F32, tag="spair", name="kvps")
                for dc in range(DC):
                    xs = xsp.tile([128, NBW], BF16, tag="xs")
                    nc.sync.dma_start(out=xs[:],
                                      in_=xT[dc * 128:(dc + 1) * 128, ncol])
                    st, sp = dc == 0, dc == DC - 1
                    for s in range(4):
                        nc.tensor.matmul(q_ps[s][:],
                                         wq_sb[dc][:, s * 128:(s + 1) * 128],
                                         xs[:], start=st, stop=sp)
                    nc.tensor.matmul(kv_ps[:, 0:NBW], wk_sb[dc][:], xs[:],
                                     start=st, stop=sp)
                    nc.tensor.matmul(kv_ps[:, NBW:2 * NBW], wv_sb[dc][:],
                                     xs[:], start=st, stop=sp)
                    if dc == 0:
                        flush_tr()
                # vts copy first: the deferred transposes depend on it
                vts = vtp.tile([128, NBW], BF16, tag="vts")
                nc.vector.tensor_copy(vts[:], kv_ps[:, NBW:2 * NBW])
                for s in range(4):
                    nc.vector.tensor_copy(qt[s][:, ncol], q_ps[s][:])
                nc.vector.tensor_copy(ktp[0][0:64, ncol],
                                      kv_ps[0:64, 0:NBW])
                nc.vector.tensor_copy(ktp[1][64:128, ncol],
                                      kv_ps[64:128, 0:NBW])

                def _tr(vts=vts, nb=nb):
                    tq = psp2.tile([128, 2 * NBW], BF16, tag="spair",
                                   name="tq")
                    for i in range(4):
                        nc.tensor.transpose(tq[:, i * 128:(i + 1) * 128],
                                            vts[:, i * 128:(i + 1) * 128],
                                            ident[:])
                    for i in range(4):
                        mt = nb * 4 + i
                        nc.vector.tensor_copy(vaug[mt][:, 0:DH],
                                              tq[:, i * 128:i * 128 + DH])
                        nc.vector.tensor_copy(
                            vaug[mt][:, VW2:VW2 + DH],
                            tq[:, i * 128 + DH:i * 128 + 2 * DH])
                nonlocal_pend_tr[0] = _tr

            nonlocal_pend_tr = [None]

            def flush_tr():
                if nonlocal_pend_tr[0] is not None:
                    nonlocal_pend_tr[0]()
                    nonlocal_pend_tr[0] = None

            # ---- attention (+ interleaved out-proj units) ----
            scale = 1.0 / np.sqrt(DH)
            norm_idx = [0]

            def emit_norm(c_ps, j, par, q0):
                # ctx^T rows /= row 64 (the ones-col sums): write the sums
                # into the zeroed row tile, broadcast to partitions 0:64
                # with the full-K ones matmul, fast reciprocal, multiply.
                # Engines cannot shift partitions, so the odd-parity half
                # goes through a small sbuf->sbuf DMA into ctxT 64:128.
                lrt = lr[norm_idx[0] % 2]
                norm_idx[0] += 1
                nc.vector.tensor_copy(lrt[DH:DH + 1, :], c_ps[DH:DH + 1, :])
                rb_ps = psp.tile([DH, NBW], F32, tag="ps", name="rbps")
                nc.tensor.matmul(rb_ps[:], onesP[:], lrt[:],
                                 start=True, stop=True)
                rb = smp.tile([DH, NBW], F32, tag="rb", name="rb")
                nc.vector.reciprocal_approx_fast(rb[:], rb_ps[:])
                if par == 0:
                    nc.vector.tensor_mul(ctxT[j][0:DH, q0:q0 + NBW],
                                         c_ps[0:DH, :], rb[:])
                else:
                    tmp = smp.tile([DH, NBW], BF16, tag="ctmp", name="ctmp")
                    nc.vector.tensor_mul(tmp[:], c_ps[0:DH, :], rb[:])
                    nc.sync.dma_start(out=ctxT[j][DH:2 * DH, q0:q0 + NBW],
                                      in_=tmp[:])

            o_sb_of = {}
            pend_ocopy = [None]

            def emit_ounit(nt, ob):
                # one (row-tile, out-block) unit of the out projection;
                # the psum->sbuf copy + store are deferred one unit so
                # attention DVE work is never queued behind them
                if nt not in o_sb_of:
                    o_sb_of[nt] = outp.tile([128, D], BF16, tag="osb",
                                            name="osb")
                o_sb = o_sb_of[nt]
                o_ps = psp.tile([128, NBW], F32, tag="ps", name="ops")
                for j in range(4):
                    nc.tensor.matmul(o_ps[:],
                                     ctxT[j][:, nt * 128:(nt + 1) * 128],
                                     wo_sb[(j, ob)][:],
                                     start=(j == 0), stop=(j == 3))
                if pend_ocopy[0] is not None:
                    pend_ocopy[0]()

                def _ocopy(o_sb=o_sb, o_ps=o_ps, nt=nt, ob=ob):
                    nc.vector.tensor_copy(o_sb[:, ob * NBW:(ob + 1) * NBW],
                                          o_ps[:])
                    nc.sync.dma_start(
                        out=OUT[nt * 128:(nt + 1) * 128,
                                ob * NBW:(ob + 1) * NBW],
                        in_=o_sb[:, ob * NBW:(ob + 1) * NBW])
                pend_ocopy[0] = _ocopy

            fin_state = [None]
            pend_units = deque()

            def flush_fin():
                if fin_state[0] is not None:
                    fin_state[0]()
                    fin_state[0] = None

            def emit_attn_block(hh, nb):
                    kv, g = hh // G, hh % G
                    j, par = hh // 2, hh % 2
                    q0 = nb * NBW
                    c_ps = psp.tile([128, NBW], F32, tag="ps", name="cps")
                    vcol = slice(kv * VW2, (kv + 1) * VW2)
                    n_mb = 4 * nb + 4
                    pend = deque()
                    for pr in range(n_mb // 2):
                        s_pair = psp2.tile([128, 2 * NBW], F32, tag="spair",
                                           name="sp")
                        widths = []
                        for half in (0, 1):
                            mb = 2 * pr + half
                            m0 = mb * 128
                            off = max(0, m0 - q0)
                            w = NBW - off
                            widths.append((mb, off, w))
                            nc.tensor.matmul(
                                s_pair[:, half * NBW:half * NBW + w],
                                ktp[kv][:, m0:m0 + 128],
                                qt[g][:, q0 + off:q0 + NBW],
                                start=True, stop=True)
                        p_pair = ptp.tile([128, 2 * NBW], BF16, tag="pt",
                                          name="pt")
                        ew = NBW + widths[1][2]
                        nc.scalar.activation(p_pair[:, 0:ew], s_pair[:, 0:ew],
                                             EXP, scale=float(scale))
                        for half in (0, 1):
                            mb, off, w = widths[half]
                            if mb >= 4 * nb:  # diagonal: triangle mask
                                nc.vector.tensor_mul(
                                    p_pair[:, half * NBW:half * NBW + 128],
                                    p_pair[:, half * NBW:half * NBW + 128],
                                    tri[:])

                        def _ctxpair(c_ps=c_ps, p_pair=p_pair, widths=widths,
                                     vcol=vcol, n_mb=n_mb):
                            for half in (0, 1):
                                mb, off, w = widths[half]
                                nc.tensor.matmul(
                                    c_ps[:, off:NBW],
                                    vaug[mb][:, vcol],
                                    p_pair[:, half * NBW:half * NBW + w],
                                    start=(mb == 0), stop=(mb == n_mb - 1))
                        pend.append(_ctxpair)
                        if pr == 1:
                            flush_fin()

                    def _fin(pend=pend, c_ps=c_ps, j=j, par=par, q0=q0):
                        while pend:
                            pend.popleft()()
                        emit_norm(c_ps, j, par, q0)
                    fin_state[0] = _fin

                    # interleave two out-proj units of the previous q-block
                    for _ in range(2):
                        if pend_units:
                            emit_ounit(*pend_units.popleft())

            # ---- schedule: projections first, then attention q-blocks
            # with the previous block's out-proj units interleaved two per
            # head-block (they fill the PE while ACT works through exps).
            for nb in range(NB):
                emit_proj(nb)
            for nb in range(NB):
                if nb == NB - 1:
                    # the last q-block's V transposes are only needed here;
                    # emitting them now costs nothing (vts long since ready)
                    flush_tr()
                for hh in range(HPC):
                    emit_attn_block(hh, nb)
                # all heads of nb emitted; its out-proj units become
                # available once the last fin lands (next block, pr==1)
                pend_units.extend((4 * nb + i, ob)
                                  for i in range(4) for ob in range(4))
            flush_fin()
            while pend_units:
                emit_ounit(*pend_units.popleft())
            if pend_ocopy[0] is not None:
                pend_ocopy[0]()

    nc.compile()
    return nc


def kernel(x, Wq, Wk, Wv, Wo, bo):
    x = np.asarray(x, dtype=np.float32)
    Wq = np.asarray(Wq, dtype=np.float32)
    Wk = np.asarray(Wk, dtype=np.float32)
    Wv = np.asarray(Wv, dtype=np.float32)
    Wo = np.asarray(Wo, dtype=np.float32)
    bo = np.asarray(bo, dtype=np.float32)

    if "nc" not in _CACHED:
        _CACHED["nc"] = _build()
    nc = _CACHED["nc"]

    xTb = [x[b].T.astype(BF) for b in range(B)]
    wk_t = [Wk[:, t * 128:(t + 1) * 128].astype(BF) for t in range(4)]
    wv_t = [Wv[:, t * 128:(t + 1) * 128].astype(BF) for t in range(4)]
    wo_t = [Wo[t * DQ:(t + 1) * DQ, :].astype(BF) for t in range(4)]
    wq_t = []
    for t in range(4):
        # q slab s holds [kv-head 2t head g=s | kv-head 2t+1 head g=s]
        qcols = []
        for s in range(4):
            for kvl in range(KVPC):
                h = (2 * t + kvl) * G + s
                qcols.append(Wq[:, h * DH:(h + 1) * DH])
        wq_t.append(np.concatenate(qcols, axis=1).astype(BF))

    in_maps = []
    for c in range(8):
        b, t = c // 4, c % 4
        in_maps.append({"xT": xTb[b], "Wq": wq_t[t], "Wk": wk_t[t],
                        "Wv": wv_t[t], "Wo": wo_t[t]})

    trace = bool(int(os.environ.get("GQA_TRACE", "0")))
    kwargs = {}
    if trace:
        import tempfile
        td = os.environ.get("GQA_TRACE_DIR") or tempfile.mkdtemp(prefix="gqa_")
        kwargs = dict(trace=True, tmpdir=td)
    res = run_bass_kernel_spmd(nc, in_maps, list(range(8)), **kwargs)
    _CACHED["last_result"] = res

    out = np.empty((B, N, D), dtype=np.float32)
    for b in range(B):
        acc = res.results[4 * b]["out"].astype(np.float32)
        for t in range(1, 4):
            acc = acc + res.results[4 * b + t]["out"].astype(np.float32)
        out[b] = acc + bo[None, :]
    return out


# revision 49
# speedup vs baseline: 1.0146x; 1.0030x over previous
"""GQA forward (B=2,N=2048,D=2048,H=32,KV=8,DH=64, causal) on 8 trn2 cores.

Sharding: 2-way data parallel over batch x 4-way tensor parallel over heads
(each core: 8 q-heads = 2 kv-heads, keeping group structure). Row-parallel
out-proj; the all-reduce over the 4 TP shards (+ bias) happens on host at
gather time.

Device kernel (per core), all PE matmuls in bf16 (PSUM accum fp32).
Half-sized matmuls stream at the slow PE clock on trn2, so every matmul is
padded to the full 128-partition contraction / 128 stationary columns:

  phase 1: streaming projections from xT (host-pretransposed bf16), input
           DMAs interleaved across queues so the first matmul starts ~2us
           in. K^T is stored per kv head with the other head's 64 rows
           zeroed (the zeros annihilate the other head's q rows, keeping
           K=128). V^T -> V via bf16 PE transposes deferred into the next
           q-block's matmul stream.
  phase 2: causal attention per head in S^T orientation, q-blocks outer,
           heads inner. Scores for two 128-key blocks share a double-bank
           psum tile and a single exp (ACT, folded 1/sqrt(dh) scale, bf16
           out). Triangle mask multiply on diagonal blocks only. ctx^T
           accumulates in psum; vaug's ones column gives the softmax
           denominator on partition 64 (cols 65:128 are pad). The
           denominator is broadcast with a full-K ones matmul against a
           persistent zeroed row tile, then fast-reciprocal + DVE
           multiply. ctx pairs trail their scores pair by two so the
           in-order PE queue never waits on ACT.
  phase 2.5: out-projection row tiles of q-block nb are interleaved two
           (nt, ob) units per head-block into q-block nb+1's attention,
           overlapping out-proj PE work with ACT exp time.
  phase 3: the final q-block's out-proj tail, stored bf16 per chunk
           (host upcasts and all-reduces the TP shards).
"""
import os
import sys
from collections import deque

import numpy as np

if "/opt/trn_rl_repo" not in sys.path:
    sys.path.insert(0, "/opt/trn_rl_repo")

import ml_dtypes

import concourse.bacc as bacc
import concourse.tile as tile
from concourse import mybir
from concourse.bass_utils import run_bass_kernel_spmd
from concourse.masks import make_identity

F32 = mybir.dt.float32
BF16 = mybir.dt.bfloat16
EXP = mybir.ActivationFunctionType.Exp
BF = ml_dtypes.bfloat16

B, N, D = 2, 2048, 2048
H, KV, DH = 32, 8, 64
G = H // KV                      # 4 q-heads per kv head
HPC, KVPC = 8, 2                 # heads / kv-heads per core
DQ = HPC * DH                    # 512 per-core q projection width
NT = N // 128                    # 16 row tiles
NBW = 512                        # q-block width for attention
NB = N // NBW                    # 4 q-blocks
DC = D // 128                    # 16 contraction chunks
VW2 = 128                        # padded vaug stride per kv head

_CACHED = {}


def _build():
    nc = bacc.Bacc("TRN2", target_bir_lowering=False, debug=False, num_devices=8)

    xT = nc.dram_tensor("xT", [D, N], BF16, kind="ExternalInput")
    Wq = nc.dram_tensor("Wq", [D, DQ], BF16, kind="ExternalInput")
    Wk = nc.dram_tensor("Wk", [D, KVPC * DH], BF16, kind="ExternalInput")
    Wv = nc.dram_tensor("Wv", [D, KVPC * DH], BF16, kind="ExternalInput")
    Wo = nc.dram_tensor("Wo", [DQ, D], BF16, kind="ExternalInput")
    OUT = nc.dram_tensor("out", [N, D], BF16, kind="ExternalOutput")

    with tile.TileContext(nc) as tc:
        with (
            tc.tile_pool(name="persist", bufs=1) as pp,
            tc.tile_pool(name="wbig", bufs=16) as wbig,
            tc.tile_pool(name="wkv", bufs=16) as wkvp,
            tc.tile_pool(name="xs", bufs=6) as xsp,
            tc.tile_pool(name="vt", bufs=2) as vtp,
            tc.tile_pool(name="pt", bufs=10) as ptp,
            tc.tile_pool(name="outs", bufs=2) as outp,
            tc.tile_pool(name="small", bufs=3) as smp,
            tc.tile_pool(name="ps", bufs=4, space="PSUM") as psp,
            tc.tile_pool(name="ps2", bufs=2, space="PSUM") as psp2,
        ):
            # ---- persistent sbuf state ----
            ident = pp.tile([128, 128], BF16, tag="ident")
            make_identity(nc, ident[:])
            # lower-triangle-in-column-sense mask: mask[r, j] = 1 if j >= r
            tri = pp.tile([128, 128], BF16, tag="tri")
            nc.gpsimd.memset(tri[:], 1.0)
            nc.gpsimd.affine_select(
                out=tri[:], in_=tri[:],
                compare_op=mybir.AluOpType.is_ge,
                fill=0.0, base=0,
                pattern=[[1, 128]],
                channel_multiplier=-1,
            )

            qt = [pp.tile([128, N], BF16, tag=f"qt{s}", name=f"qt{s}")
                  for s in range(4)]
            # K^T per kv head padded to the full 128 contraction rows; the
            # other kv head's 64 rows stay zero.
            ktp = [pp.tile([128, N], BF16, tag=f"kt{v}", name=f"kt{v}")
                   for v in range(KVPC)]
            for v in range(KVPC):
                nc.vector.memset(ktp[v][:], 0.0)
            # vaug (stride VW2=128 per kv head): [V (64) | ones (64)]; col 64
            # puts the softmax row-sum on psum partition 64 of the ctx
            # matmul, cols 65:128 pad the stationary to 128 (their psum
            # partitions are never read). memset once; only V columns get
            # overwritten.
            vaug = [pp.tile([128, KVPC * VW2], BF16, tag=f"va{m}", name=f"va{m}")
                    for m in range(NT)]
            for m in range(NT):
                nc.vector.memset(vaug[m][:], 1.0)
            ctxT = [pp.tile([128, N], BF16, tag=f"ct{j}", name=f"ct{j}")
                    for j in range(4)]
            # full-K denominator broadcast: onesP row 64 is ones, the lr
            # tiles are zero everywhere except row 64 (written per norm),
            # so the K=128 ones matmul reproduces the K=1 broadcast at the
            # fast clock with no garbage terms.
            onesP = pp.tile([128, DH], BF16, tag="onesP")
            nc.vector.memset(onesP[:], 0.0)
            nc.vector.memset(onesP[DH:DH + 1, :], 1.0)
            lr = [pp.tile([128, NBW], BF16, tag=f"lr{i}", name=f"lr{i}")
                  for i in range(2)]
            for i in range(2):
                nc.vector.memset(lr[i][:], 0.0)

            # ---- weight loads spread over the idle queues (and their
            # DMA engines): wk on scalar, wv+wq interleaved on gpsimd,
            # wo after them; the x stream has sync to itself ----
            wk_sb, wv_sb, wq_sb = [], [], []
            for dc in range(DC):
                t = wkvp.tile([128, KVPC * DH], BF16, tag="wk")
                nc.scalar.dma_start(out=t[:], in_=Wk[dc * 128:(dc + 1) * 128, :])
                wk_sb.append(t)
            for dc in range(DC):
                t = wkvp.tile([128, KVPC * DH], BF16, tag="wv")
                nc.gpsimd.dma_start(out=t[:], in_=Wv[dc * 128:(dc + 1) * 128, :])
                wv_sb.append(t)
                t = wbig.tile([128, DQ], BF16, tag="w")
                nc.gpsimd.dma_start(out=t[:], in_=Wq[dc * 128:(dc + 1) * 128, :])
                wq_sb.append(t)
            wo_sb = {}
            for j in range(4):
                for ob in range(4):
                    t = wbig.tile([128, NBW], BF16, tag="wo")
                    nc.gpsimd.dma_start(
                        out=t[:],
                        in_=Wo[j * 128:(j + 1) * 128, ob * NBW:(ob + 1) * NBW])
                    wo_sb[(j, ob)] = t

            def emit_proj(nb):
                ncol = slice(nb * NBW, (nb + 1) * NBW)
                q_ps = [psp.tile([128, NBW], F32, tag="ps", name=f"qps{_}")
                        for _ in range(4)]
                kv_ps = psp2.tile([128, 2 * NBW], F32, tag="spair", name="kvps")
                for dc in range(DC):
                    xs = xsp.tile([128, NBW], BF16, tag="xs")
                    nc.sync.dma_start(out=xs[:],
                                      in_=xT[dc * 128:(dc + 1) * 128, ncol])
                    st, sp = dc == 0, dc == DC - 1
                    for s in range(4):
                        nc.tensor.matmul(q_ps[s][:],
                                         wq_sb[dc][:, s * 128:(s + 1) * 128],
                                         xs[:], start=st, stop=sp)
                    nc.tensor.matmul(kv_ps[:, 0:NBW], wk_sb[dc][:], xs[:],
                                     start=st, stop=sp)
                    nc.tensor.matmul(kv_ps[:, NBW:2 * NBW], wv_sb[dc][:],
                                     xs[:], start=st, stop=sp)
                    if dc == 0:
                        flush_tr()
                # vts copy first: the deferred transposes depend on it
                vts = vtp.tile([128, NBW], BF16, tag="vts")
                nc.vector.tensor_copy(vts[:], kv_ps[:, NBW:2 * NBW])
                for s in range(4):
                    nc.vector.tensor_copy(qt[s][:, ncol], q_ps[s][:])
                nc.vector.tensor_copy(ktp[0][0:64, ncol],
                                      kv_ps[0:64, 0:NBW])
                nc.vector.tensor_copy(ktp[1][64:128, ncol],
                                      kv_ps[64:128, 0:NBW])

                def _tr(vts=vts, nb=nb):
                    tq = psp2.tile([128, 2 * NBW], BF16, tag="spair",
                                   name="tq")
                    for i in range(4):
                        nc.tensor.transpose(tq[:, i * 128:(i + 1) * 128],
                                            vts[:, i * 128:(i + 1) * 128],
                                            ident[:])
                    for i in range(4):
                        mt = nb * 4 + i
                        nc.vector.tensor_copy(vaug[mt][:, 0:DH],
                                              tq[:, i * 128:i * 128 + DH])
                        nc.vector.tensor_copy(
                            vaug[mt][:, VW2:VW2 + DH],
                            tq[:, i * 128 + DH:i * 128 + 2 * DH])
                nonlocal_pend_tr[0] = _tr

            nonlocal_pend_tr = [None]

            def flush_tr():
                if nonlocal_pend_tr[0] is not None:
                    nonlocal_pend_tr[0]()
                    nonlocal_pend_tr[0] = None

            # ---- attention (+ interleaved out-proj units) ----
            scale = 1.0 / np.sqrt(DH)
            norm_idx = [0]

            def emit_norm(c_ps, j, par, q0):
                # ctx^T rows /= row 64 (the ones-col sums): write the sums
                # into the zeroed row tile, broadcast to partitions 0:64
                # with the full-K ones matmul, fast reciprocal, multiply.
                # Engines cannot shift partitions, so the odd-parity half
                # goes through a small sbuf->sbuf DMA into ctxT 64:128.
                lrt = lr[norm_idx[0] % 2]
                norm_idx[0] += 1
                nc.vector.tensor_copy(lrt[DH:DH + 1, :], c_ps[DH:DH + 1, :])
                rb_ps = psp.tile([DH, NBW], F32, tag="ps", name="rbps")
                nc.tensor.matmul(rb_ps[:], onesP[:], lrt[:],
                                 start=True, stop=True)
                rb = smp.tile([DH, NBW], F32, tag="rb", name="rb")
                nc.vector.reciprocal_approx_fast(rb[:], rb_ps[:])
                if par == 0:
                    nc.vector.tensor_mul(ctxT[j][0:DH, q0:q0 + NBW],
                                         c_ps[0:DH, :], rb[:])
                else:
                    tmp = smp.tile([DH, NBW], BF16, tag="ctmp", name="ctmp")
                    nc.vector.tensor_mul(tmp[:], c_ps[0:DH, :], rb[:])
                    nc.sync.dma_start(out=ctxT[j][DH:2 * DH, q0:q0 + NBW],
                                      in_=tmp[:])

            o_sb_of = {}
            pend_ocopy = [None]

            def emit_ounit(nt, ob):
                # one (row-tile, out-block) unit of the out projection;
                # the psum->sbuf copy + store are deferred one unit so
                # attention DVE work is never queued behind them
                if nt not in o_sb_of:
                    o_sb_of[nt] = outp.tile([128, D], BF16, tag="osb",
                                            name="osb")
                o_sb = o_sb_of[nt]
                o_ps = psp.tile([128, NBW], F32, tag="ps", name="ops")
                for j in range(4):
                    nc.tensor.matmul(o_ps[:],
                                     ctxT[j][:, nt * 128:(nt + 1) * 128],
                                     wo_sb[(j, ob)][:],
                                     start=(j == 0), stop=(j == 3))
                if pend_ocopy[0] is not None:
                    pend_ocopy[0]()

                def _ocopy(o_sb=o_sb, o_ps=o_ps, nt=nt, ob=ob):
                    nc.vector.tensor_copy(o_sb[:, ob * NBW:(ob + 1) * NBW],
                                          o_ps[:])
                    nc.sync.dma_start(
                        out=OUT[nt * 128:(nt + 1) * 128,
                                ob * NBW:(ob + 1) * NBW],
                        in_=o_sb[:, ob * NBW:(ob + 1) * NBW])
                pend_ocopy[0] = _ocopy

            fin_state = [None]
            pend_units = deque()

            def flush_fin():
                if fin_state[0] is not None:
                    fin_state[0]()
                    fin_state[0] = None

            def emit_attn_block(hh, nb):
                    kv, g = hh // G, hh % G
                    j, par = hh // 2, hh % 2
                    q0 = nb * NBW
                    c_ps = psp.tile([128, NBW], F32, tag="ps", name="cps")
                    vcol = slice(kv * VW2, (kv + 1) * VW2)
                    n_mb = 4 * nb + 4
                    pend = deque()
                    for pr in range(n_mb // 2):
                        s_pair = psp2.tile([128, 2 * NBW], F32, tag="spair",
                                           name="sp")
                        widths = []
                        base = 0
                        for half in (0, 1):
                            mb = 2 * pr + half
                            m0 = mb * 128
                            off = max(0, m0 - q0)
                            w = NBW - off
                            # pack the two blocks adjacently so the exp
                            # covers no garbage columns
                            widths.append((mb, off, w, base))
                            nc.tensor.matmul(
                                s_pair[:, base:base + w],
                                ktp[kv][:, m0:m0 + 128],
                                qt[g][:, q0 + off:q0 + NBW],
                                start=True, stop=True)
                            base += w
                        p_pair = ptp.tile([128, 2 * NBW], BF16, tag="pt",
                                          name="pt")
                        nc.scalar.activation(p_pair[:, 0:base],
                                             s_pair[:, 0:base],
                                             EXP, scale=float(scale))
                        for half in (0, 1):
                            mb, off, w, b0 = widths[half]
                            if mb >= 4 * nb:  # diagonal: triangle mask
                                nc.vector.tensor_mul(
                                    p_pair[:, b0:b0 + 128],
                                    p_pair[:, b0:b0 + 128],
                                    tri[:])

                        def _ctxpair(c_ps=c_ps, p_pair=p_pair, widths=widths,
                                     vcol=vcol, n_mb=n_mb):
                            for half in (0, 1):
                                mb, off, w, b0 = widths[half]
                                nc.tensor.matmul(
                                    c_ps[:, off:NBW],
                                    vaug[mb][:, vcol],
                                    p_pair[:, b0:b0 + w],
                                    start=(mb == 0), stop=(mb == n_mb - 1))
                        pend.append(_ctxpair)
                        if pr == 1:
                            flush_fin()

                    def _fin(pend=pend, c_ps=c_ps, j=j, par=par, q0=q0):
                        while pend:
                            pend.popleft()()
                        emit_norm(c_ps, j, par, q0)
                    fin_state[0] = _fin

                    # interleave two out-proj units of the previous q-block
                    for _ in range(2):
                        if pend_units:
                            emit_ounit(*pend_units.popleft())

            # ---- schedule: projections first, then attention q-blocks
            # with the previous block's out-proj units interleaved two per
            # head-block (they fill the PE while ACT works through exps).
            for nb in range(NB):
                emit_proj(nb)
            for nb in range(NB):
                if nb == NB - 1:
                    # the last q-block's V transposes are only needed here;
                    # emitting them now costs nothing (vts long since ready)
                    flush_tr()
                for hh in range(HPC):
                    emit_attn_block(hh, nb)
                # all heads of nb emitted; its out-proj units become
                # available once the last fin lands (next block, pr==1)
                pend_units.extend((4 * nb + i, ob)
                                  for i in range(4) for ob in range(4))
            flush_fin()
            while pend_units:
                emit_ounit(*pend_units.popleft())
            if pend_ocopy[0] is not None:
                pend_ocopy[0]()

    nc.compile()
    return nc


def kernel(x, Wq, Wk, Wv, Wo, bo):
    x = np.asarray(x, dtype=np.float32)
    Wq = np.asarray(Wq, dtype=np.float32)
    Wk = np.asarray(Wk, dtype=np.float32)
    Wv = np.asarray(Wv, dtype=np.float32)
    Wo = np.asarray(Wo, dtype=np.float32)
    bo = np.asarray(bo, dtype=np.float32)

    if "nc" not in _CACHED:
        _CACHED["nc"] = _build()
    nc = _CACHED["nc"]

    xTb = [x[b].T.astype(BF) for b in range(B)]
    wk_t = [Wk[:, t * 128:(t + 1) * 128].astype(BF) for t in range(4)]
    wv_t = [Wv[:, t * 128:(t + 1) * 128].astype(BF) for t in range(4)]
    wo_t = [Wo[t * DQ:(t + 1) * DQ, :].astype(BF) for t in range(4)]
    wq_t = []
    for t in range(4):
        # q slab s holds [kv-head 2t head g=s | kv-head 2t+1 head g=s]
        qcols = []
        for s in range(4):
            for kvl in range(KVPC):
                h = (2 * t + kvl) * G + s
                qcols.append(Wq[:, h * DH:(h + 1) * DH])
        wq_t.append(np.concatenate(qcols, axis=1).astype(BF))

    in_maps = []
    for c in range(8):
        b, t = c // 4, c % 4
        in_maps.append({"xT": xTb[b], "Wq": wq_t[t], "Wk": wk_t[t],
                        "Wv": wv_t[t], "Wo": wo_t[t]})

    trace = bool(int(os.environ.get("GQA_TRACE", "0")))
    kwargs = {}
    if trace:
        import tempfile
        td = os.environ.get("GQA_TRACE_DIR") or tempfile.mkdtemp(prefix="gqa_")
        kwargs = dict(trace=True, tmpdir=td)
    res = run_bass_kernel_spmd(nc, in_maps, list(range(8)), **kwargs)
    _CACHED["last_result"] = res

    out = np.empty((B, N, D), dtype=np.float32)
    for b in range(B):
        acc = res.results[4 * b]["out"].astype(np.float32)
        for t in range(1, 4):
            acc = acc + res.results[4 * b + t]["out"].astype(np.float32)
        out[b] = acc + bo[None, :]
    return out


# revision 50
# speedup vs baseline: 1.0179x; 1.0033x over previous
"""GQA forward (B=2,N=2048,D=2048,H=32,KV=8,DH=64, causal) on 8 trn2 cores.

Sharding: 2-way data parallel over batch x 4-way tensor parallel over heads
(each core: 8 q-heads = 2 kv-heads, keeping group structure). Row-parallel
out-proj; the all-reduce over the 4 TP shards (+ bias) happens on host at
gather time.

Device kernel (per core), all PE matmuls in bf16 (PSUM accum fp32).
Half-sized matmuls stream at the slow PE clock on trn2, so every matmul is
padded to the full 128-partition contraction / 128 stationary columns:

  phase 1: streaming projections from xT (host-pretransposed bf16), input
           DMAs interleaved across queues so the first matmul starts ~2us
           in. K^T is stored per kv head with the other head's 64 rows
           zeroed (the zeros annihilate the other head's q rows, keeping
           K=128). V^T -> V via bf16 PE transposes deferred into the next
           q-block's matmul stream.
  phase 2: causal attention per head in S^T orientation, q-blocks outer,
           heads inner. Scores for two 128-key blocks share a double-bank
           psum tile and a single exp (ACT, folded 1/sqrt(dh) scale, bf16
           out). Triangle mask multiply on diagonal blocks only. ctx^T
           accumulates in psum; vaug's ones column gives the softmax
           denominator on partition 64 (cols 65:128 are pad). The
           denominator is broadcast with a full-K ones matmul against a
           persistent zeroed row tile, then fast-reciprocal + DVE
           multiply. ctx pairs trail their scores pair by two so the
           in-order PE queue never waits on ACT.
  phase 2.5: out-projection row tiles of q-block nb are interleaved two
           (nt, ob) units per head-block into q-block nb+1's attention,
           overlapping out-proj PE work with ACT exp time.
  phase 3: the final q-block's out-proj tail, stored bf16 per chunk
           (host upcasts and all-reduces the TP shards).
"""
import os
import sys
from collections import deque

import numpy as np

if "/opt/trn_rl_repo" not in sys.path:
    sys.path.insert(0, "/opt/trn_rl_repo")

import ml_dtypes

import concourse.bacc as bacc
import concourse.tile as tile
from concourse import mybir
from concourse.bass_utils import run_bass_kernel_spmd
from concourse.masks import make_identity

F32 = mybir.dt.float32
BF16 = mybir.dt.bfloat16
EXP = mybir.ActivationFunctionType.Exp
BF = ml_dtypes.bfloat16

B, N, D = 2, 2048, 2048
H, KV, DH = 32, 8, 64
G = H // KV                      # 4 q-heads per kv head
HPC, KVPC = 8, 2                 # heads / kv-heads per core
DQ = HPC * DH                    # 512 per-core q projection width
NT = N // 128                    # 16 row tiles
NBW = 512                        # q-block width for attention
NB = N // NBW                    # 4 q-blocks
DC = D // 128                    # 16 contraction chunks
VW2 = 128                        # padded vaug stride per kv head

_CACHED = {}


def _build():
    nc = bacc.Bacc("TRN2", target_bir_lowering=False, debug=False, num_devices=8)

    xT = nc.dram_tensor("xT", [D, N], BF16, kind="ExternalInput")
    Wq = nc.dram_tensor("Wq", [D, DQ], BF16, kind="ExternalInput")
    Wk = nc.dram_tensor("Wk", [D, KVPC * DH], BF16, kind="ExternalInput")
    Wv = nc.dram_tensor("Wv", [D, KVPC * DH], BF16, kind="ExternalInput")
    Wo = nc.dram_tensor("Wo", [DQ, D], BF16, kind="ExternalInput")
    OUT = nc.dram_tensor("out", [N, D], BF16, kind="ExternalOutput")

    with tile.TileContext(nc) as tc:
        with (
            tc.tile_pool(name="persist", bufs=1) as pp,
            tc.tile_pool(name="wbig", bufs=16) as wbig,
            tc.tile_pool(name="wkv", bufs=16) as wkvp,
            tc.tile_pool(name="xs", bufs=6) as xsp,
            tc.tile_pool(name="vt", bufs=2) as vtp,
            tc.tile_pool(name="pt", bufs=10) as ptp,
            tc.tile_pool(name="outs", bufs=2) as outp,
            tc.tile_pool(name="small", bufs=3) as smp,
            tc.tile_pool(name="ps", bufs=4, space="PSUM") as psp,
            tc.tile_pool(name="ps2", bufs=2, space="PSUM") as psp2,
        ):
            # ---- persistent sbuf state ----
            ident = pp.tile([128, 128], BF16, tag="ident")
            make_identity(nc, ident[:])
            # lower-triangle-in-column-sense mask: mask[r, j] = 1 if j >= r
            tri = pp.tile([128, 128], BF16, tag="tri")
            nc.gpsimd.memset(tri[:], 1.0)
            nc.gpsimd.affine_select(
                out=tri[:], in_=tri[:],
                compare_op=mybir.AluOpType.is_ge,
                fill=0.0, base=0,
                pattern=[[1, 128]],
                channel_multiplier=-1,
            )

            qt = [pp.tile([128, N], BF16, tag=f"qt{s}", name=f"qt{s}")
                  for s in range(4)]
            # K^T per kv head padded to the full 128 contraction rows; the
            # other kv head's 64 rows stay zero.
            ktp = [pp.tile([128, N], BF16, tag=f"kt{v}", name=f"kt{v}")
                   for v in range(KVPC)]
            for v in range(KVPC):
                nc.vector.memset(ktp[v][:], 0.0)
            # vaug (stride VW2=128 per kv head): [V (64) | ones (64)]; col 64
            # puts the softmax row-sum on psum partition 64 of the ctx
            # matmul, cols 65:128 pad the stationary to 128 (their psum
            # partitions are never read). memset once; only V columns get
            # overwritten.
            vaug = [pp.tile([128, KVPC * VW2], BF16, tag=f"va{m}", name=f"va{m}")
                    for m in range(NT)]
            for m in range(NT):
                nc.vector.memset(vaug[m][:], 1.0)
            ctxT = [pp.tile([128, N], BF16, tag=f"ct{j}", name=f"ct{j}")
                    for j in range(4)]
            # full-K denominator broadcast: onesP row 64 is ones, the lr
            # tiles are zero everywhere except row 64 (written per norm),
            # so the K=128 ones matmul reproduces the K=1 broadcast at the
            # fast clock with no garbage terms.
            onesP = pp.tile([128, DH], BF16, tag="onesP")
            nc.vector.memset(onesP[:], 0.0)
            nc.vector.memset(onesP[DH:DH + 1, :], 1.0)
            # warm the ACT exp table during the projection phase so the
            # first attention exp doesn't pay the lazy table load
            warm = pp.tile([1, 8], F32, tag="warm")
            nc.scalar.activation(warm[:], onesP[0:1, 0:8], EXP)
            lr = [pp.tile([128, NBW], BF16, tag=f"lr{i}", name=f"lr{i}")
                  for i in range(2)]
            for i in range(2):
                nc.vector.memset(lr[i][:], 0.0)

            # ---- weight loads spread over the idle queues (and their
            # DMA engines): wk on scalar, wv+wq interleaved on gpsimd,
            # wo after them; the x stream has sync to itself ----
            wk_sb, wv_sb, wq_sb = [], [], []
            for dc in range(DC):
                t = wkvp.tile([128, KVPC * DH], BF16, tag="wk")
                nc.scalar.dma_start(out=t[:], in_=Wk[dc * 128:(dc + 1) * 128, :])
                wk_sb.append(t)
            for dc in range(DC):
                t = wkvp.tile([128, KVPC * DH], BF16, tag="wv")
                nc.gpsimd.dma_start(out=t[:], in_=Wv[dc * 128:(dc + 1) * 128, :])
                wv_sb.append(t)
                t = wbig.tile([128, DQ], BF16, tag="w")
                nc.gpsimd.dma_start(out=t[:], in_=Wq[dc * 128:(dc + 1) * 128, :])
                wq_sb.append(t)
            wo_sb = {}
            for j in range(4):
                for ob in range(4):
                    t = wbig.tile([128, NBW], BF16, tag="wo")
                    nc.gpsimd.dma_start(
                        out=t[:],
                        in_=Wo[j * 128:(j + 1) * 128, ob * NBW:(ob + 1) * NBW])
                    wo_sb[(j, ob)] = t

            def emit_proj(nb):
                ncol = slice(nb * NBW, (nb + 1) * NBW)
                q_ps = [psp.tile([128, NBW], F32, tag="ps", name=f"qps{_}")
                        for _ in range(4)]
                kv_ps = psp2.tile([128, 2 * NBW], F32, tag="spair", name="kvps")
                for dc in range(DC):
                    xs = xsp.tile([128, NBW], BF16, tag="xs")
                    nc.sync.dma_start(out=xs[:],
                                      in_=xT[dc * 128:(dc + 1) * 128, ncol])
                    st, sp = dc == 0, dc == DC - 1
                    for s in range(4):
                        nc.tensor.matmul(q_ps[s][:],
                                         wq_sb[dc][:, s * 128:(s + 1) * 128],
                                         xs[:], start=st, stop=sp)
                    nc.tensor.matmul(kv_ps[:, 0:NBW], wk_sb[dc][:], xs[:],
                                     start=st, stop=sp)
                    nc.tensor.matmul(kv_ps[:, NBW:2 * NBW], wv_sb[dc][:],
                                     xs[:], start=st, stop=sp)
                    if dc == 0:
                        flush_tr()
                # vts copy first: the deferred transposes depend on it
                vts = vtp.tile([128, NBW], BF16, tag="vts")
                nc.vector.tensor_copy(vts[:], kv_ps[:, NBW:2 * NBW])
                for s in range(4):
                    nc.vector.tensor_copy(qt[s][:, ncol], q_ps[s][:])
                nc.vector.tensor_copy(ktp[0][0:64, ncol],
                                      kv_ps[0:64, 0:NBW])
                nc.vector.tensor_copy(ktp[1][64:128, ncol],
                                      kv_ps[64:128, 0:NBW])

                def _tr(vts=vts, nb=nb):
                    tq = psp2.tile([128, 2 * NBW], BF16, tag="spair",
                                   name="tq")
                    for i in range(4):
                        nc.tensor.transpose(tq[:, i * 128:(i + 1) * 128],
                                            vts[:, i * 128:(i + 1) * 128],
                                            ident[:])
                    for i in range(4):
                        mt = nb * 4 + i
                        nc.vector.tensor_copy(vaug[mt][:, 0:DH],
                                              tq[:, i * 128:i * 128 + DH])
                        nc.vector.tensor_copy(
                            vaug[mt][:, VW2:VW2 + DH],
                            tq[:, i * 128 + DH:i * 128 + 2 * DH])
                nonlocal_pend_tr[0] = _tr

            nonlocal_pend_tr = [None]

            def flush_tr():
                if nonlocal_pend_tr[0] is not None:
                    nonlocal_pend_tr[0]()
                    nonlocal_pend_tr[0] = None

            # ---- attention (+ interleaved out-proj units) ----
            scale = 1.0 / np.sqrt(DH)
            norm_idx = [0]

            def emit_norm(c_ps, j, par, q0):
                # ctx^T rows /= row 64 (the ones-col sums): write the sums
                # into the zeroed row tile, broadcast to partitions 0:64
                # with the full-K ones matmul, fast reciprocal, multiply.
                # Engines cannot shift partitions, so the odd-parity half
                # goes through a small sbuf->sbuf DMA into ctxT 64:128.
                lrt = lr[norm_idx[0] % 2]
                norm_idx[0] += 1
                nc.vector.tensor_copy(lrt[DH:DH + 1, :], c_ps[DH:DH + 1, :])
                rb_ps = psp.tile([DH, NBW], F32, tag="ps", name="rbps")
                nc.tensor.matmul(rb_ps[:], onesP[:], lrt[:],
                                 start=True, stop=True)
                rb = smp.tile([DH, NBW], F32, tag="rb", name="rb")
                nc.vector.reciprocal_approx_fast(rb[:], rb_ps[:])
                if par == 0:
                    nc.vector.tensor_mul(ctxT[j][0:DH, q0:q0 + NBW],
                                         c_ps[0:DH, :], rb[:])
                else:
                    tmp = smp.tile([DH, NBW], BF16, tag="ctmp", name="ctmp")
                    nc.vector.tensor_mul(tmp[:], c_ps[0:DH, :], rb[:])
                    nc.sync.dma_start(out=ctxT[j][DH:2 * DH, q0:q0 + NBW],
                                      in_=tmp[:])

            o_sb_of = {}
            pend_ocopy = [None]

            def emit_ounit(nt, ob):
                # one (row-tile, out-block) unit of the out projection;
                # the psum->sbuf copy + store are deferred one unit so
                # attention DVE work is never queued behind them
                if nt not in o_sb_of:
                    o_sb_of[nt] = outp.tile([128, D], BF16, tag="osb",
                                            name="osb")
                o_sb = o_sb_of[nt]
                o_ps = psp.tile([128, NBW], F32, tag="ps", name="ops")
                for j in range(4):
                    nc.tensor.matmul(o_ps[:],
                                     ctxT[j][:, nt * 128:(nt + 1) * 128],
                                     wo_sb[(j, ob)][:],
                                     start=(j == 0), stop=(j == 3))
                if pend_ocopy[0] is not None:
                    pend_ocopy[0]()

                def _ocopy(o_sb=o_sb, o_ps=o_ps, nt=nt, ob=ob):
                    nc.vector.tensor_copy(o_sb[:, ob * NBW:(ob + 1) * NBW],
                                          o_ps[:])
                    nc.sync.dma_start(
                        out=OUT[nt * 128:(nt + 1) * 128,
                                ob * NBW:(ob + 1) * NBW],
                        in_=o_sb[:, ob * NBW:(ob + 1) * NBW])
                pend_ocopy[0] = _ocopy

            fin_state = [None]
            pend_units = deque()

            def flush_fin():
                if fin_state[0] is not None:
                    fin_state[0]()
                    fin_state[0] = None

            def emit_attn_block(hh, nb):
                    kv, g = hh // G, hh % G
                    j, par = hh // 2, hh % 2
                    q0 = nb * NBW
                    c_ps = psp.tile([128, NBW], F32, tag="ps", name="cps")
                    vcol = slice(kv * VW2, (kv + 1) * VW2)
                    n_mb = 4 * nb + 4
                    pend = deque()
                    for pr in range(n_mb // 2):
                        s_pair = psp2.tile([128, 2 * NBW], F32, tag="spair",
                                           name="sp")
                        widths = []
                        base = 0
                        for half in (0, 1):
                            mb = 2 * pr + half
                            m0 = mb * 128
                            off = max(0, m0 - q0)
                            w = NBW - off
                            # pack the two blocks adjacently so the exp
                            # covers no garbage columns
                            widths.append((mb, off, w, base))
                            nc.tensor.matmul(
                                s_pair[:, base:base + w],
                                ktp[kv][:, m0:m0 + 128],
                                qt[g][:, q0 + off:q0 + NBW],
                                start=True, stop=True)
                            base += w
                        p_pair = ptp.tile([128, 2 * NBW], BF16, tag="pt",
                                          name="pt")
                        nc.scalar.activation(p_pair[:, 0:base],
                                             s_pair[:, 0:base],
                                             EXP, scale=float(scale))
                        for half in (0, 1):
                            mb, off, w, b0 = widths[half]
                            if mb >= 4 * nb:  # diagonal: triangle mask
                                nc.vector.tensor_mul(
                                    p_pair[:, b0:b0 + 128],
                                    p_pair[:, b0:b0 + 128],
                                    tri[:])

                        def _ctxpair(c_ps=c_ps, p_pair=p_pair, widths=widths,
                                     vcol=vcol, n_mb=n_mb):
                            for half in (0, 1):
                                mb, off, w, b0 = widths[half]
                                nc.tensor.matmul(
                                    c_ps[:, off:NBW],
                                    vaug[mb][:, vcol],
                                    p_pair[:, b0:b0 + w],
                                    start=(mb == 0), stop=(mb == n_mb - 1))
                        pend.append(_ctxpair)
                        if pr == 1:
                            flush_fin()

                    def _fin(pend=pend, c_ps=c_ps, j=j, par=par, q0=q0):
                        while pend:
                            pend.popleft()()
                        emit_norm(c_ps, j, par, q0)
                    fin_state[0] = _fin

                    # interleave two out-proj units of the previous q-block
                    for _ in range(2):
                        if pend_units:
                            emit_ounit(*pend_units.popleft())

            # ---- schedule: projections first, then attention q-blocks
            # with the previous block's out-proj units interleaved two per
            # head-block (they fill the PE while ACT works through exps).
            for nb in range(NB):
                emit_proj(nb)
            for nb in range(NB):
                if nb == NB - 1:
                    # the last q-block's V transposes are only needed here;
                    # emitting them now costs nothing (vts long since ready)
                    flush_tr()
                for hh in range(HPC):
                    emit_attn_block(hh, nb)
                # all heads of nb emitted; its out-proj units become
                # available once the last fin lands (next block, pr==1)
                pend_units.extend((4 * nb + i, ob)
                                  for i in range(4) for ob in range(4))
            flush_fin()
            while pend_units:
                emit_ounit(*pend_units.popleft())
            if pend_ocopy[0] is not None:
                pend_ocopy[0]()

    nc.compile()
    return nc


def kernel(x, Wq, Wk, Wv, Wo, bo):
    x = np.asarray(x, dtype=np.float32)
    Wq = np.asarray(Wq, dtype=np.float32)
    Wk = np.asarray(Wk, dtype=np.float32)
    Wv = np.asarray(Wv, dtype=np.float32)
    Wo = np.asarray(Wo, dtype=np.float32)
    bo = np.asarray(bo, dtype=np.float32)

    if "nc" not in _CACHED:
        _CACHED["nc"] = _build()
    nc = _CACHED["nc"]

    xTb = [x[b].T.astype(BF) for b in range(B)]
    wk_t = [Wk[:, t * 128:(t + 1) * 128].astype(BF) for t in range(4)]
    wv_t = [Wv[:, t * 128:(t + 1) * 128].astype(BF) for t in range(4)]
    wo_t = [Wo[t * DQ:(t + 1) * DQ, :].astype(BF) for t in range(4)]
    wq_t = []
    for t in range(4):
        # q slab s holds [kv-head 2t head g=s | kv-head 2t+1 head g=s]
        qcols = []
        for s in range(4):
            for kvl in range(KVPC):
                h = (2 * t + kvl) * G + s
                qcols.append(Wq[:, h * DH:(h + 1) * DH])
        wq_t.append(np.concatenate(qcols, axis=1).astype(BF))

    in_maps = []
    for c in range(8):
        b, t = c // 4, c % 4
        in_maps.append({"xT": xTb[b], "Wq": wq_t[t], "Wk": wk_t[t],
                        "Wv": wv_t[t], "Wo": wo_t[t]})

    trace = bool(int(os.environ.get("GQA_TRACE", "0")))
    kwargs = {}
    if trace:
        import tempfile
        td = os.environ.get("GQA_TRACE_DIR") or tempfile.mkdtemp(prefix="gqa_")
        kwargs = dict(trace=True, tmpdir=td)
    res = run_bass_kernel_spmd(nc, in_maps, list(range(8)), **kwargs)
    _CACHED["last_result"] = res

    out = np.empty((B, N, D), dtype=np.float32)
    for b in range(B):
        acc = res.results[4 * b]["out"].astype(np.float32)
        for t in range(1, 4):
            acc = acc + res.results[4 * b + t]["out"].astype(np.float32)
        out[b] = acc + bo[None, :]
    return out
